# revision 39
# baseline (speedup 1.0000x reference)
"""Chamfer loss kernel for Trainium2 (8 NeuronCores, Bass/Tile).

Problem: x [4, 8192, 3], y [4, 8192, 3] float32.
  d2[b,n,m] = ||x[b,n] - y[b,m]||^2
  out = mean_b,n(min_m d2) + mean_b,m(min_n d2)   (scalar float32)

Strategy
--------
Sharding: 8 cores = 4 batches x 2 halves of the N axis. Core c handles
batch c//2, x-rows [4096*(c%2), 4096*(c%2+1)), full y of that batch.

Per core the 4096x8192 distance matrix is produced tile-by-tile on the
TensorEngine with an augmented K=5 fp32 matmul (rows: -2x^T, x^2, 1
against y, 1, y^2), so each PSUM tile [128n, 512m] holds exact-fp32 d2
values without any extra elementwise work. ScalarE converts each PSUM
group to fp16 in SBUF (enabling the DVE 2x perf mode), and VectorE does
two fp16 min passes flash-style, never materializing d2 in HBM:
  - colmin: running elementwise min over n-tiles -> [128, 8192]
  - rowmin: running elementwise min over m within an n-tile -> [128, 1]
The tiny cross-partition / cross-core tails are finished on the host.
"""

import numpy as np

try:
    import concourse.bass as bass
except ImportError:  # pragma: no cover - environment fallback
    import sys

    sys.path.insert(0, "/opt/trn_rl_repo")
    import concourse.bass as bass

import concourse.bacc as bacc
import concourse.mybir as mybir
import concourse.tile as tile
from concourse.bass_utils import run_bass_kernel_spmd

P = 128  # SBUF/PSUM partitions
MM_FREE = 512  # matmul moving-operand free dim (one PSUM bank of fp32 out)
M_GROUP = 2048  # PSUM group: 4 banks converted by one ACT instruction
RM_W = 1024  # rowmin accumulator width
K_AUG = 24  # augmented contraction rows (see make_core_inputs)

N_CORES = 8
B, N, D = 4, 8192, 3
N_PER_CORE = N // 2  # 4096 x-rows per core
M_PER_CORE = N  # full y per core


def build_nc(
    n_rows: int = N_PER_CORE, m_cols: int = M_PER_CORE, reps: int = 1
) -> bass.Bass:
    """Build the per-core Bass program (SPMD: same program, per-core data).

    reps > 1 wraps the compute body in a hardware For_i loop repeating the
    identical (idempotent) computation — used only for differential wall
    clock timing of the on-device execution.
    """
    assert n_rows % P == 0 and m_cols % M_GROUP == 0
    n_tiles = n_rows // P
    n_groups = m_cols // M_GROUP

    # Bacc (not raw Bass): its compile() pipeline runs
    # move_matmul_waits_to_ldweights + generate_event_semaphores, which
    # split multi-wait instructions to satisfy the TRN2 1-wait limit.
    nc = bacc.Bacc("TRN2", target_bir_lowering=False, debug=True)
    lhs_d = nc.dram_tensor(
        "lhs", [K_AUG, n_rows], mybir.dt.bfloat16, kind="ExternalInput"
    )
    rhs_d = nc.dram_tensor(
        "rhs", [K_AUG, m_cols], mybir.dt.bfloat16, kind="ExternalInput"
    )
    colmin_d = nc.dram_tensor(
        "colmin", [P, m_cols], mybir.dt.float16, kind="ExternalOutput"
    )
    # 1024-wide per-tile rowmin rows; the final min over the last axis is
    # finished on the host (saves the 1x-rate DVE reduce + last fold level)
    rowmin_d = nc.dram_tensor(
        "rowmin", [P, n_tiles, RM_W], mybir.dt.float16, kind="ExternalOutput"
    )

    f32 = mybir.dt.float32
    f16 = mybir.dt.float16
    bf16 = mybir.dt.bfloat16
    amin = mybir.AluOpType.min

    with tile.TileContext(nc) as tc:
        with (
            tc.tile_pool(name="singles", bufs=1) as singles,
            tc.tile_pool(name="psum", bufs=2, space="PSUM") as psum_pool,
            tc.tile_pool(name="conv", bufs=8) as conv_pool,
            tc.tile_pool(name="rm", bufs=3) as rm_pool,
        ):
            lhs_sb = singles.tile([K_AUG, n_rows], bf16)
            rhs_sb = singles.tile([K_AUG, m_cols], bf16)
            acc = singles.tile([P, m_cols], f16)  # colmin accumulator
            rm_all = singles.tile([P, n_tiles, RM_W], f16)  # per-tile rowmins
            # Both input DMAs on the single SWDGE queue -> one semaphore, so
            # the first (weight-self-loading) matmul carries only ONE wait:
            # walrus's S3_LW slot rejects matmuls with >1 sync wait.
            nc.gpsimd.dma_start(out=lhs_sb, in_=lhs_d[:])
            nc.gpsimd.dma_start(out=rhs_sb, in_=rhs_d[:])

            def body():
                half = n_tiles // 2
                for i in range(n_tiles):
                    # per-group 1024-wide rowmin leaves (independent ops)
                    rm = rm_pool.tile([P, n_groups, RM_W], f16)
                    lhsT = lhs_sb[:, i * P : (i + 1) * P]
                    for g in range(n_groups):
                        ps = psum_pool.tile([P, M_GROUP], f32)
                        for k in range(M_GROUP // MM_FREE):
                            off = g * M_GROUP + k * MM_FREE
                            nc.tensor.matmul(
                                ps[:, k * MM_FREE : (k + 1) * MM_FREE],
                                lhsT,
                                rhs_sb[:, off : off + MM_FREE],
                                start=True,
                                stop=True,
                            )
                        if i == 0:
                            # first n-tile initializes the colmin accumulator
                            conv = acc[:, g * M_GROUP : (g + 1) * M_GROUP]
                            nc.scalar.copy(out=conv, in_=ps)
                        else:
                            conv = conv_pool.tile([P, M_GROUP], f16)
                            nc.scalar.copy(out=conv, in_=ps)
                            accs = acc[:, g * M_GROUP : (g + 1) * M_GROUP]
                            nc.vector.tensor_tensor(accs, conv, accs, amin)
                        # independent per-group rowmin leaf: 2048 -> 1024
                        nc.vector.tensor_tensor(
                            rm[:, g, :], conv[:, 0:RM_W], conv[:, RM_W : 2 * RM_W],
                            amin,
                        )
                    # pairwise fold of the leaves; last merge lands in rm_all
                    stride = 1
                    while stride < n_groups // 2:
                        for g in range(0, n_groups, 2 * stride):
                            if g + stride < n_groups:
                                nc.vector.tensor_tensor(
                                    rm[:, g, :], rm[:, g, :], rm[:, g + stride, :],
                                    amin,
                                )
                        stride *= 2
                    if n_groups == 1:
                        nc.vector.tensor_copy(rm_all[:, i, :], rm[:, 0, :])
                    else:
                        nc.vector.tensor_tensor(
                            rm_all[:, i, :], rm[:, 0, :], rm[:, n_groups // 2, :],
                            amin,
                        )
                    if i == half - 1:
                        # stream out the first half early (DMA engines idle)
                        nc.sync.dma_start(
                            out=rowmin_d[:, 0:half, :], in_=rm_all[:, 0:half, :]
                        )
                nc.sync.dma_start(
                    out=rowmin_d[:, half:, :], in_=rm_all[:, half:, :]
                )
                for g in range(n_groups):
                    nc.sync.dma_start(
                        out=colmin_d[:, g * M_GROUP : (g + 1) * M_GROUP],
                        in_=acc[:, g * M_GROUP : (g + 1) * M_GROUP],
                    )

            if reps == 1:
                body()
            else:
                with tc.For_i(0, reps, 1):
                    body()
    # Run the Bacc compile pipeline (register allocation, matmul-wait
    # splitting, event semaphores) — walrus rejects the raw form.
    nc.finalize()
    return nc


def _split3(v64: np.ndarray):
    """3-way bf16 split: v ~= h + m + l with residual ~2^-27 relative."""
    import ml_dtypes

    bf = ml_dtypes.bfloat16
    h = v64.astype(bf)
    r1 = v64 - h.astype(np.float64)
    m = r1.astype(bf)
    r2 = r1 - m.astype(np.float64)
    l = r2.astype(bf)
    return h, m, l


def make_core_inputs(xs: np.ndarray, ys: np.ndarray) -> dict[str, np.ndarray]:
    """Augmented bf16 matmul operands so one K=24 matmul yields fp32-grade d2.

    d2[n,m] = |x_n|^2 + |y_m|^2 - 2<x_n,y_m>. The PE runs bf16 at 4x the
    fp32 rate, so each fp32 value is split 3-way into bf16 limbs
    (x = xh+xm+xl, residual ~2^-27): the -2<x,y> term uses limb pairs
    (h,h),(h,m),(m,h),(h,l),(l,h),(m,m) per coordinate (18 rows), and
    |x|^2 / |y|^2 are 3-way-split against ones (6 rows). All products are
    exact in the PE's fp32 PSUM accumulation; dropped terms are ~2^-27.
    K does not affect PE cycles (free-dim bound), so the 24 rows are free.
    """
    import ml_dtypes

    bf = ml_dtypes.bfloat16
    n, m = xs.shape[0], ys.shape[0]
    x64 = xs.astype(np.float64)
    y64 = ys.astype(np.float64)
    xh, xm, xl = _split3(x64)  # [n, 3] each
    yh, ym, yl = _split3(y64)  # [m, 3]
    x2h, x2m, x2l = _split3(np.einsum("nd,nd->n", x64, x64))
    y2h, y2m, y2l = _split3(np.einsum("md,md->m", y64, y64))

    ones_n = np.ones(n, bf)
    ones_m = np.ones(m, bf)
    n2 = np.float64(-2.0)

    lhs_rows, rhs_rows = [], []
    # -2<x,y> limb pairs; scaling by -2 is exact in bf16.
    for xa, yb in ((xh, yh), (xh, ym), (xm, yh), (xh, yl), (xl, yh), (xm, ym)):
        for d in range(3):
            lhs_rows.append((n2 * xa[:, d].astype(np.float64)).astype(bf))
            rhs_rows.append(yb[:, d])
    for row in (x2h, x2m, x2l):
        lhs_rows.append(row)
        rhs_rows.append(ones_m)
    for row in (y2h, y2m, y2l):
        lhs_rows.append(ones_n)
        rhs_rows.append(row)

    lhs = np.stack(lhs_rows).astype(bf)  # [24, n]
    rhs = np.stack(rhs_rows).astype(bf)  # [24, m]
    assert lhs.shape == (K_AUG, n) and rhs.shape == (K_AUG, m)
    return {"lhs": lhs, "rhs": rhs}


_NC_CACHE: dict[tuple[int, int, int], bass.Bass] = {}


def _get_nc(n_rows: int, m_cols: int, reps: int = 1) -> bass.Bass:
    key = (n_rows, m_cols, reps)
    if key not in _NC_CACHE:
        _NC_CACHE[key] = build_nc(n_rows, m_cols, reps)
    return _NC_CACHE[key]


def kernel(x: np.ndarray, y: np.ndarray) -> np.ndarray:
    x = np.asarray(x, dtype=np.float32)
    y = np.asarray(y, dtype=np.float32)
    assert x.shape == (B, N, D) and y.shape == (B, N, D), (x.shape, y.shape)

    nc = _get_nc(N_PER_CORE, M_PER_CORE)
    in_maps = []
    for c in range(N_CORES):
        b, h = divmod(c, 2)
        xs = x[b, h * N_PER_CORE : (h + 1) * N_PER_CORE]
        ys = y[b]
        in_maps.append(make_core_inputs(xs, ys))

    results = run_bass_kernel_spmd(nc, in_maps, list(range(N_CORES))).results

    # Host tails (tiny): cross-partition min, cross-core min, means.
    min_xy_sum = 0.0
    min_yx = np.full((B, N), np.inf, dtype=np.float64)
    for c in range(N_CORES):
        b = c // 2
        # [128, n_tiles, 1024] -> finish the last fold on the host
        rowmin = results[c]["rowmin"].astype(np.float32).min(axis=2)
        min_xy_sum += rowmin.astype(np.float64).sum()
        colmin = results[c]["colmin"].astype(np.float64)  # [128, m]
        min_yx[b] = np.minimum(min_yx[b], colmin.min(axis=0))
    mean_xy = min_xy_sum / (B * N)
    mean_yx = min_yx.mean()
    return np.asarray(mean_xy + mean_yx, dtype=np.float32)


# revision 43
# speedup vs baseline: 1.4527x; 1.4527x over previous
"""Chamfer loss kernel for Trainium2 (8 NeuronCores, Bass/Tile).

Problem: x [4, 8192, 3], y [4, 8192, 3] float32.
  d2[b,n,m] = ||x[b,n] - y[b,m]||^2
  out = mean_b,n(min_m d2) + mean_b,m(min_n d2)   (scalar float32)

Strategy
--------
Sharding: 8 cores = 4 batches x 2 halves of the N axis. Core c handles
batch c//2, x-rows [4096*(c%2), 4096*(c%2+1)), full y of that batch.

Per core the 4096x8192 distance matrix is produced tile-by-tile on the
TensorEngine with an augmented K=5 fp32 matmul (rows: -2x^T, x^2, 1
against y, 1, y^2), so each PSUM tile [128n, 512m] holds exact-fp32 d2
values without any extra elementwise work. ScalarE converts each PSUM
group to fp16 in SBUF (enabling the DVE 2x perf mode), and VectorE does
two fp16 min passes flash-style, never materializing d2 in HBM:
  - colmin: running elementwise min over n-tiles -> [128, 8192]
  - rowmin: running elementwise min over m within an n-tile -> [128, 1]
The tiny cross-partition / cross-core tails are finished on the host.
"""

import numpy as np

try:
    import concourse.bass as bass
except ImportError:  # pragma: no cover - environment fallback
    import sys

    sys.path.insert(0, "/opt/trn_rl_repo")
    import concourse.bass as bass

import concourse.bacc as bacc
import concourse.mybir as mybir
import concourse.tile as tile
from concourse.bass_utils import run_bass_kernel_spmd

P = 128  # SBUF/PSUM partitions
MM_FREE = 512  # matmul moving-operand free dim (one PSUM bank of fp32 out)
M_GROUP = 2048  # PSUM group: 4 banks converted by one ACT instruction
RM_W = 1024  # rowmin accumulator width
K_AUG = 24  # augmented contraction rows (see make_core_inputs)

N_CORES = 8
B, N, D = 4, 8192, 3
N_PER_CORE = N // 2  # 4096 x-rows per core
M_PER_CORE = N  # full y per core


def build_nc(
    n_rows: int = N_PER_CORE, m_cols: int = M_PER_CORE, reps: int = 1
) -> bass.Bass:
    """Build the per-core Bass program (SPMD: same program, per-core data).

    reps > 1 wraps the compute body in a hardware For_i loop repeating the
    identical (idempotent) computation — used only for differential wall
    clock timing of the on-device execution.
    """
    assert n_rows % P == 0 and m_cols % M_GROUP == 0
    n_tiles = n_rows // P
    n_groups = m_cols // M_GROUP

    # Bacc (not raw Bass): its compile() pipeline runs
    # move_matmul_waits_to_ldweights + generate_event_semaphores, which
    # split multi-wait instructions to satisfy the TRN2 1-wait limit.
    nc = bacc.Bacc("TRN2", target_bir_lowering=False, debug=True)
    lhs_d = nc.dram_tensor(
        "lhs", [K_AUG, n_rows], mybir.dt.bfloat16, kind="ExternalInput"
    )
    rhs_d = nc.dram_tensor(
        "rhs", [K_AUG, m_cols], mybir.dt.bfloat16, kind="ExternalInput"
    )
    colmin_d = nc.dram_tensor(
        "colmin", [P, m_cols], mybir.dt.float16, kind="ExternalOutput"
    )
    rowmin_d = nc.dram_tensor(
        "rowmin", [P, n_tiles], mybir.dt.float16, kind="ExternalOutput"
    )

    f32 = mybir.dt.float32
    f16 = mybir.dt.float16
    bf16 = mybir.dt.bfloat16
    amin = mybir.AluOpType.min

    with tile.TileContext(nc) as tc:
        with (
            tc.tile_pool(name="singles", bufs=1) as singles,
            tc.tile_pool(name="psum", bufs=2, space="PSUM") as psum_pool,
            tc.tile_pool(name="conv", bufs=8) as conv_pool,
            tc.tile_pool(name="rm", bufs=3) as rm_pool,
        ):
            lhs_sb = singles.tile([K_AUG, n_rows], bf16)
            rhs_sb = singles.tile([K_AUG, m_cols], bf16)
            acc = singles.tile([P, m_cols], f16)  # colmin accumulator
            rm_all = singles.tile([P, n_tiles, 512], f16)  # per-tile rowmins
            rm_out = singles.tile([P, n_tiles], f16)
            # Both input DMAs on the single SWDGE queue -> one semaphore, so
            # the first (weight-self-loading) matmul carries only ONE wait:
            # walrus's S3_LW slot rejects matmuls with >1 sync wait.
            nc.gpsimd.dma_start(out=lhs_sb, in_=lhs_d[:])
            nc.gpsimd.dma_start(out=rhs_sb, in_=rhs_d[:])

            def body():
                half = n_tiles // 2
                for i in range(n_tiles):
                    # per-group 1024-wide rowmin leaves (independent ops)
                    rm = rm_pool.tile([P, n_groups, RM_W], f16)
                    lhsT = lhs_sb[:, i * P : (i + 1) * P]
                    for g in range(n_groups):
                        ps = psum_pool.tile([P, M_GROUP], f32)
                        for k in range(M_GROUP // MM_FREE):
                            off = g * M_GROUP + k * MM_FREE
                            nc.tensor.matmul(
                                ps[:, k * MM_FREE : (k + 1) * MM_FREE],
                                lhsT,
                                rhs_sb[:, off : off + MM_FREE],
                                start=True,
                                stop=True,
                            )
                        if i == 0:
                            # first n-tile initializes the colmin accumulator
                            conv = acc[:, g * M_GROUP : (g + 1) * M_GROUP]
                            nc.scalar.copy(out=conv, in_=ps)
                        else:
                            conv = conv_pool.tile([P, M_GROUP], f16)
                            nc.scalar.copy(out=conv, in_=ps)
                            accs = acc[:, g * M_GROUP : (g + 1) * M_GROUP]
                            nc.vector.tensor_tensor(accs, conv, accs, amin)
                        # independent per-group rowmin leaf: 2048 -> 1024
                        nc.vector.tensor_tensor(
                            rm[:, g, :], conv[:, 0:RM_W], conv[:, RM_W : 2 * RM_W],
                            amin,
                        )
                    # shallow pairwise fold of the leaves -> one 1024 row
                    stride = 1
                    while stride < n_groups:
                        for g in range(0, n_groups, 2 * stride):
                            if g + stride < n_groups:
                                nc.vector.tensor_tensor(
                                    rm[:, g, :], rm[:, g, :], rm[:, g + stride, :],
                                    amin,
                                )
                        stride *= 2
                    nc.vector.tensor_tensor(
                        rm_all[:, i, :], rm[:, 0, 0:512], rm[:, 0, 512:1024], amin
                    )
                    if i == half - 1:
                        nc.vector.tensor_reduce(
                            rm_out[:, 0:half],
                            rm_all[:, 0:half, :],
                            axis=mybir.AxisListType.X,
                            op=amin,
                        )
                nc.vector.tensor_reduce(
                    rm_out[:, half:],
                    rm_all[:, half:, :],
                    axis=mybir.AxisListType.X,
                    op=amin,
                )
                for g in range(n_groups):
                    nc.sync.dma_start(
                        out=colmin_d[:, g * M_GROUP : (g + 1) * M_GROUP],
                        in_=acc[:, g * M_GROUP : (g + 1) * M_GROUP],
                    )
                nc.sync.dma_start(out=rowmin_d[:], in_=rm_out)

            if reps == 1:
                body()
            else:
                with tc.For_i(0, reps, 1):
                    body()
    # Run the Bacc compile pipeline (register allocation, matmul-wait
    # splitting, event semaphores) — walrus rejects the raw form.
    nc.finalize()
    return nc


def _split3(v64: np.ndarray):
    """3-way bf16 split: v ~= h + m + l with residual ~2^-27 relative."""
    import ml_dtypes

    bf = ml_dtypes.bfloat16
    h = v64.astype(bf)
    r1 = v64 - h.astype(np.float64)
    m = r1.astype(bf)
    r2 = r1 - m.astype(np.float64)
    l = r2.astype(bf)
    return h, m, l


def make_core_inputs(xs: np.ndarray, ys: np.ndarray) -> dict[str, np.ndarray]:
    """Augmented bf16 matmul operands so one K=24 matmul yields fp32-grade d2.

    d2[n,m] = |x_n|^2 + |y_m|^2 - 2<x_n,y_m>. The PE runs bf16 at 4x the
    fp32 rate, so each fp32 value is split 3-way into bf16 limbs
    (x = xh+xm+xl, residual ~2^-27): the -2<x,y> term uses limb pairs
    (h,h),(h,m),(m,h),(h,l),(l,h),(m,m) per coordinate (18 rows), and
    |x|^2 / |y|^2 are 3-way-split against ones (6 rows). All products are
    exact in the PE's fp32 PSUM accumulation; dropped terms are ~2^-27.
    K does not affect PE cycles (free-dim bound), so the 24 rows are free.
    """
    import ml_dtypes

    bf = ml_dtypes.bfloat16
    n, m = xs.shape[0], ys.shape[0]
    x64 = xs.astype(np.float64)
    y64 = ys.astype(np.float64)
    xh, xm, xl = _split3(x64)  # [n, 3] each
    yh, ym, yl = _split3(y64)  # [m, 3]
    x2h, x2m, x2l = _split3(np.einsum("nd,nd->n", x64, x64))
    y2h, y2m, y2l = _split3(np.einsum("md,md->m", y64, y64))

    ones_n = np.ones(n, bf)
    ones_m = np.ones(m, bf)
    n2 = np.float64(-2.0)

    lhs_rows, rhs_rows = [], []
    # -2<x,y> limb pairs; scaling by -2 is exact in bf16.
    for xa, yb in ((xh, yh), (xh, ym), (xm, yh), (xh, yl), (xl, yh), (xm, ym)):
        for d in range(3):
            lhs_rows.append((n2 * xa[:, d].astype(np.float64)).astype(bf))
            rhs_rows.append(yb[:, d])
    for row in (x2h, x2m, x2l):
        lhs_rows.append(row)
        rhs_rows.append(ones_m)
    for row in (y2h, y2m, y2l):
        lhs_rows.append(ones_n)
        rhs_rows.append(row)

    lhs = np.stack(lhs_rows).astype(bf)  # [24, n]
    rhs = np.stack(rhs_rows).astype(bf)  # [24, m]
    assert lhs.shape == (K_AUG, n) and rhs.shape == (K_AUG, m)
    return {"lhs": lhs, "rhs": rhs}


_NC_CACHE: dict[tuple[int, int, int], bass.Bass] = {}


def _get_nc(n_rows: int, m_cols: int, reps: int = 1) -> bass.Bass:
    key = (n_rows, m_cols, reps)
    if key not in _NC_CACHE:
        _NC_CACHE[key] = build_nc(n_rows, m_cols, reps)
    return _NC_CACHE[key]


def kernel(x: np.ndarray, y: np.ndarray) -> np.ndarray:
    x = np.asarray(x, dtype=np.float32)
    y = np.asarray(y, dtype=np.float32)
    assert x.shape == (B, N, D) and y.shape == (B, N, D), (x.shape, y.shape)

    nc = _get_nc(N_PER_CORE, M_PER_CORE)
    in_maps = []
    for c in range(N_CORES):
        b, h = divmod(c, 2)
        xs = x[b, h * N_PER_CORE : (h + 1) * N_PER_CORE]
        ys = y[b]
        in_maps.append(make_core_inputs(xs, ys))

    results = run_bass_kernel_spmd(nc, in_maps, list(range(N_CORES))).results

    # Host tails (tiny): cross-partition min, cross-core min, means.
    min_xy_sum = 0.0
    min_yx = np.full((B, N), np.inf, dtype=np.float64)
    for c in range(N_CORES):
        b = c // 2
        rowmin = results[c]["rowmin"].astype(np.float64)  # [128, n_tiles]
        min_xy_sum += rowmin.sum()
        colmin = results[c]["colmin"].astype(np.float64)  # [128, m]
        min_yx[b] = np.minimum(min_yx[b], colmin.min(axis=0))
    mean_xy = min_xy_sum / (B * N)
    mean_yx = min_yx.mean()
    return np.asarray(mean_xy + mean_yx, dtype=np.float32)
